# revision 37
# baseline (speedup 1.0000x reference)
"""Channel-attention (XCA) block on 8 trn2 NeuronCores, data-parallel over batch.

Per core: x (4096, 768) -> qkv -> per-head channel attention (96x96 scores over
l2-normalized q,k transposed to (Ch, N)) -> proj.  All big matmuls run in bf16
with fp32 PSUM accumulation; norms/softmax in fp32.

v7: the whole output path is algebraically folded into a single GEMM:
    y = x @ W2 + b,   W2 = Wv . Q,   Q = blockdiag(attn_h^T) . Wproj
(valid because channel attention applies a fixed 96x96 mixing matrix per head,
so attn commutes past the token dimension).  attn_h feeds the Q-build matmul
directly as the stationary operand -- no transposes, no (Ch x N) value tensor,
no (768 x 4096) attention output in SBUF.  Remaining structure: merged-q|k qkv
matmuls (N=384) off a reordered Wqkv layout, sum-of-squares norms on the Pool
engine with one matmul per group, softmax via a single ln/exp activation table
(no table reloads), bf16 PE x-transposes behind ACT casts, and a deferred
PE-op queue pacing softmax-dependent matmuls into the next group's qk blocks.
"""

import numpy as np
from contextlib import ExitStack

import bass_rust
import concourse.bass as bass
import concourse.tile as tile
from concourse import mybir
from concourse.masks import make_identity
from concourse.bass_utils import run_bass_kernel_spmd

F32 = mybir.dt.float32
BF = mybir.dt.bfloat16
AF = mybir.ActivationFunctionType

P = 128          # partitions
N = 4096         # tokens per core (batch element)
C = 768          # channels
H = 8            # heads
CH = 96          # channels per head
KC = C // P      # 6 contraction chunks of 128
NB = N // P      # 32 token blocks of 128
G = 4            # head groups
HPG = H // G     # 2 heads per group
GC = HPG * CH    # 192 q (or k) columns per group
EPS2 = 1e-24     # eps^2 clamp on sum-of-squares (torch F.normalize eps=1e-12)
LAG = 10         # qk blocks trail the transpose loop by this many blocks


def build_nc():
    nc = bass.Bass()

    x_d = nc.dram_tensor("x", [N, C], F32, kind="ExternalInput")
    wqkv_d = nc.dram_tensor("Wqkv", [C, 3 * C], F32, kind="ExternalInput")
    temp_d = nc.dram_tensor("temperature", [H], F32, kind="ExternalInput")
    wproj_d = nc.dram_tensor("Wproj", [C, C], F32, kind="ExternalInput")
    bproj_d = nc.dram_tensor("bproj", [C], F32, kind="ExternalInput")
    y_d = nc.dram_tensor("y", [N, C], F32, kind="ExternalOutput")

    with ExitStack() as ctx:
        tc = ctx.enter_context(tile.TileContext(nc))
        persist = ctx.enter_context(tc.tile_pool(name="persist", bufs=1))

        # persistent SBUF: xT[c%128, c//128, n] = x[n, c]  (bf16)
        xT = persist.tile([P, KC, N], BF)
        # Wv^T per head/chunk: wvT[d, kc, h, j] = Wqkv[kc*128+j, 2C + h*96 + d]
        wvT = persist.tile([CH, KC, H, P], BF)
        # Wproj rows by head: wp96[c, h, jo] = Wproj[h*96 + c, jo]
        wp96 = persist.tile([CH, H, C], BF)
        # Q[d, h, jo] = sum_c attn_h[c, d] Wproj[h*96+c, jo]
        q_sb = persist.tile([CH, H, C], BF)
        bias_sb = persist.tile([P, C], F32)

        ident128b = persist.tile([P, P], BF)
        make_identity(nc, ident128b)
        ident128f = persist.tile([P, P], F32)
        make_identity(nc, ident128f)
        ones_colf = persist.tile([P, 1], F32)    # norm-matmul lhsT (K=128, M=1)
        nc.vector.memset(ones_colf, 1.0)
        ones_row = persist.tile([1, P], BF)      # bias-matmul lhsT (K=1, M=128)
        nc.vector.memset(ones_row, 1.0)
        one1 = persist.tile([1, 1], F32)         # row->col matmul rhs
        nc.vector.memset(one1, 1.0)
        ones96 = persist.tile([1, CH], F32)
        nc.vector.memset(ones96, 1.0)

        temp_sb = persist.tile([1, H], F32)
        bstage = persist.tile([1, C], F32)
        bstage_bf = persist.tile([1, C], BF)

        # qk-phase pools on the RIGHT allocation stack so they release while
        # the (left) softmax pools stay open -- pool release is LIFO per side.
        qkctx = ctx.enter_context(ExitStack())
        # reordered q|k weights: wqk[c%128, c//128, g, 0:192]=q cols of group g,
        # [.., 192:384]=k cols (so one N=384 matmul makes q|k per block/chunk)
        wqk_pool = qkctx.enter_context(tc.tile_pool(name="wqk", bufs=1, side="right"))
        wqk = wqk_pool.tile([P, KC, G, 2 * GC], BF)
        qk_pool = qkctx.enter_context(tc.tile_pool(name="qk", bufs=1, side="right"))
        qkps = qkctx.enter_context(tc.tile_pool(name="qkps", bufs=3, space="PSUM",
                                                side="right"))
        nrmps = qkctx.enter_context(tc.tile_pool(name="nrmps", bufs=1, space="PSUM",
                                                 side="right"))
        naccpool = qkctx.enter_context(tc.tile_pool(name="nacc", bufs=2, side="right"))

        # softmax-phase pools (left): close before the W2/GEMM pools open
        softctx = ctx.enter_context(ExitStack())
        small = softctx.enter_context(tc.tile_pool(name="small", bufs=2))

        # deferred PE-op queue: closures emitted one per qk block so matmuls
        # whose operands come off the DVE/ACT softmax chain never stall the
        # in-order PE.
        deferred = []

        def emit_deferred(k=1):
            for _ in range(k):
                if deferred:
                    deferred.pop(0)()

        def qk_block(g, nb, qk_sb, nacc, in_phase_a=False):
            """merged q|k qkv matmul for one token block + eviction.

            Sum-of-squares: square from PSUM on ACT (on Pool from SBUF during
            phase A, where ACT is busy with x casts), running add on Pool.
            The PE sees one norm matmul per group, at the end of scores."""
            qkp = qkps.tile([P, 2 * GC], F32, tag="qkp")
            for kc in range(KC):
                nc.tensor.matmul(
                    qkp, xT[:, kc, nb * P:(nb + 1) * P], wqk[:, kc, g, :],
                    start=(kc == 0), stop=(kc == KC - 1))
            emit_deferred()
            qks = qk_sb[:, nb, :]
            nc.vector.tensor_copy(qks, qkp)
            sqt = naccpool.tile([P, 2 * GC], F32, tag="sqt", bufs=3)
            if in_phase_a and nb % 4 == 0:
                # ACT is busy with x casts in phase A: put a quarter of the
                # squares on Pool (from the SBUF copy), the rest on ACT (from
                # PSUM) -- Pool must drain its add chain by scores(0)'s end
                nc.gpsimd.tensor_mul(sqt, qks, qks)
            else:
                nc.scalar.activation(sqt, qkp, AF.Square)
            if nb == 0:
                nc.gpsimd.tensor_copy(nacc, sqt)
            else:
                nc.gpsimd.tensor_add(nacc, nacc, sqt)

        # per-head attn tiles passed between deferred closures
        hstate = {}

        def group_scores(g, qk_sb, nacc):
            """scores for both heads of group g (one PSUM bank, one
            interleaved accumulation group); the per-group norm matmul at the
            end (by then the Pool sum-of-squares chain has drained), with the
            rqk row via ln/exp (stays inside the one activation table set)."""
            s_all = sps.tile([CH, HPG, CH], F32, tag="s")
            for nb in range(NB):
                for hh in range(HPG):
                    nc.tensor.matmul(
                        s_all[:, hh, :],
                        qk_sb[:, nb, hh * CH:(hh + 1) * CH],
                        qk_sb[:, nb, GC + hh * CH: GC + (hh + 1) * CH],
                        start=(nb == 0 and hh == 0),
                        stop=(nb == NB - 1 and hh == HPG - 1))
            rqk = small.tile([1, 2 * GC], F32, tag="rqk")
            nqk = nrmps.tile([1, 2 * GC], F32, tag="nqk")
            nc.tensor.matmul(nqk, ones_colf, nacc, start=True, stop=True)
            # 1/max(sqrt(v), eps) = exp(-0.5 * ln(max(v, eps^2)))
            vv = small.tile([1, 2 * GC], F32, tag="vv")
            nc.vector.tensor_scalar_max(vv, nqk, EPS2)
            lnv = small.tile([1, 2 * GC], F32, tag="lnv")
            nc.scalar.activation(lnv, vv, AF.Ln)
            nc.scalar.activation(rqk, lnv, AF.Exp, scale=-0.5)

            for hh in range(HPG):
                h = g * HPG + hh
                deferred.append(lambda hh=hh, h=h, rqk=rqk, s_all=s_all:
                                head_norm_softmax(hh, h, rqk, s_all))
            # spacer between the Q-builds so head 1's matmul never waits on
            # head 0's (96, 768) PSUM eviction (qps has one rotating buffer)
            deferred.append(lambda h=g * HPG: head_qbuild(h))
            deferred.append(lambda: None)
            deferred.append(lambda h=g * HPG + 1: head_qbuild(h))

        def head_norm_softmax(hh, h, rqk, s_all):
            # norm-derived tiles (tiny PE matmuls) + softmax chain on DVE/ACT
            rq_ps = tinyps.tile([CH, 1], F32, tag="tp")
            nc.tensor.matmul(rq_ps, rqk[0:1, hh * CH:(hh + 1) * CH], one1,
                             start=True, stop=True)
            rq_col = small.tile([CH, 1], F32, tag="rqc")
            nc.vector.tensor_copy(rq_col, rq_ps)
            tempb = small.tile([1, CH], F32, tag="tb")
            nc.scalar.activation(tempb, ones96, AF.Copy,
                                 scale=temp_sb[0:1, h:h + 1])
            r_ps = tinyps.tile([CH, CH], F32, tag="tp")
            nc.tensor.matmul(r_ps, tempb,
                             rqk[0:1, GC + hh * CH: GC + (hh + 1) * CH],
                             start=True, stop=True)
            r_sb = small.tile([CH, CH], F32, tag="rsb")
            nc.vector.tensor_copy(r_sb, r_ps)

            z_sb = small.tile([CH, CH], F32, tag="z")
            nc.vector.tensor_mul(z_sb, s_all[:, hh, :], r_sb)
            e_sb = small.tile([CH, CH], BF, tag="e")
            sume = small.tile([CH, 1], F32, tag="se")
            nc.scalar.activation(e_sb, z_sb, AF.Exp,
                                 scale=rq_col, accum_out=sume)
            rden = small.tile([CH, 1], F32, tag="rd")
            nc.vector.reciprocal(rden, sume)
            attn_s = small.tile([CH, CH], BF, tag="at")
            nc.scalar.activation(attn_s, e_sb, AF.Copy, scale=rden)
            hstate[h] = attn_s

        def head_qbuild(h):
            # Q[d, h, :] = sum_c attn_h[c, d] Wproj[h*96+c, :]; attn is the
            # stationary operand directly (c on partitions) -- no transpose
            attn_s = hstate[h]
            qp = qps.tile([CH, C], F32, tag="qp")
            nc.tensor.matmul(qp[:, 0:512], attn_s, wp96[:, h, 0:512],
                             start=True, stop=True)
            nc.tensor.matmul(qp[:, 512:C], attn_s, wp96[:, h, 512:C],
                             start=True, stop=True)
            nc.vector.tensor_copy(q_sb[:, h, :], qp)

        def load_wp(h):
            st = wpstage.tile([CH, C], F32, tag="wpst")
            nc.sync.dma_start(out=st, in_=wproj_d[h * CH:(h + 1) * CH, :])
            nc.vector.tensor_copy(wp96[:, h, :], st)

        def build_bias(half):
            a, b = (0, 384) if half == 0 else (384, C)
            bps = tinyps.tile([P, 384], F32, tag="tp")
            nc.tensor.matmul(bps, ones_row, bstage_bf[0:1, a:b],
                             start=True, stop=True)
            nc.vector.tensor_copy(bias_sb[:, a:b], bps)

        # ---- Phase A: x -> xT (bf16 transposes, paired-block DMAs), Wqkv
        # load/reorder (q|k chunks first, v chunks later, on the Activation
        # hwdge ring), WvT build, fused with group-0 qk blocks.
        qk0 = qk_pool.tile([P, NB, 2 * GC], BF, tag="qk")
        nacc0 = naccpool.tile([P, 2 * GC], F32, tag="nacc")
        with tc.tile_pool(name="xstage", bufs=4) as xstage, \
             tc.tile_pool(name="xbstage", bufs=3) as xbstage, \
             tc.tile_pool(name="wstage", bufs=2) as wstage, \
             tc.tile_pool(name="wvstage", bufs=2) as wvstage, \
             tc.tile_pool(name="tps", bufs=2, space="PSUM") as tps, \
             tc.tile_pool(name="wvtps", bufs=1, space="PSUM") as wvtps:
            sv_tiles = []
            for nb in range(NB + LAG):
                if nb < NB:
                    # single-block x DMAs: each is one fully contiguous 384KB
                    # HBM read (the paired/rearranged variant alternates +-384KB
                    # between descriptors and halves effective DMA bandwidth)
                    xt_ = xstage.tile([P, C], F32, tag="x")
                    # every 4th block rides the Activation ring: the two hwdge
                    # rings stream concurrently, and the sync ring alone caps
                    # well below the core's aggregate HBM read bandwidth
                    eng = nc.scalar if nb % 4 == 3 else nc.sync
                    eng.dma_start(out=xt_, in_=x_d[nb * P:(nb + 1) * P, :])
                    xb = xbstage.tile([P, C], BF, tag="xb")
                    nc.scalar.activation(xb, xt_, AF.Copy)
                    tall = tps.tile([P, KC, P], BF, tag="t")
                    for kc in range(KC):
                        nc.tensor.matmul(tall[:, kc, :], xb[:, kc * P:(kc + 1) * P],
                                         ident128b, is_transpose=True,
                                         start=(kc == 0), stop=(kc == KC - 1))
                    nc.vector.tensor_copy(xT[:, :, nb * P:(nb + 1) * P], tall)
                if 1 <= nb <= KC:
                    # q|k columns of one Wqkv row-chunk, reordered per-group
                    kc = nb - 1
                    st = wstage.tile([P, 2, G, GC], F32, tag="wst")
                    nc.scalar.dma_start(out=st,
                                        in_=wqkv_d[kc * P:(kc + 1) * P, 0:2 * C])
                    nc.vector.tensor_copy(wqk[:, kc, :, 0:GC], st[:, 0, :, :])
                    nc.vector.tensor_copy(wqk[:, kc, :, GC:2 * GC], st[:, 1, :, :])
                if 10 <= nb <= 15:
                    # v columns arrive after all q|k chunks
                    kc = nb - 10
                    sv = wvstage.tile([P, G, GC], F32, tag="wsv", bufs=3)
                    nc.scalar.dma_start(out=sv,
                                        in_=wqkv_d[kc * P:(kc + 1) * P, 2 * C:3 * C])
                    sv_tiles.append(sv)
                if 12 <= nb <= 17:
                    # WvT via PE transposes, trailing the v DMAs
                    kc = nb - 12
                    wvtp = wvtps.tile([CH, H, P], F32, tag="wvt")
                    for h in range(H):
                        nc.tensor.matmul(
                            wvtp[:, h, :],
                            sv_tiles[kc][:, h // 2, (h % 2) * CH:(h % 2) * CH + CH],
                            ident128f, is_transpose=True,
                            start=(h in (0, 4)), stop=(h in (3, 7)))
                    nc.vector.tensor_copy(wvT[:, kc, :, :], wvtp)
                if nb == KC + 1:
                    nc.sync.dma_start(out=temp_sb,
                                      in_=temp_d.rearrange("(a h) -> a h", a=1))
                    nc.sync.dma_start(out=bstage,
                                      in_=bproj_d.rearrange("(a c) -> a c", a=1))
                    nc.vector.tensor_copy(bstage_bf, bstage)
                if nb >= LAG:
                    qk_block(0, nb - LAG, qk0, nacc0, in_phase_a=True)

        # scores/softmax PSUM pools open after the phase-A PSUM pools close
        sps = softctx.enter_context(tc.tile_pool(name="sps", bufs=1, space="PSUM"))
        tinyps = softctx.enter_context(tc.tile_pool(name="tinyps", bufs=1, space="PSUM"))
        qps = softctx.enter_context(tc.tile_pool(name="qps", bufs=1, space="PSUM"))
        wpstage = softctx.enter_context(tc.tile_pool(name="wpstage", bufs=2))

        # Wproj head-rows 0-1 must land before group 0's Q-builds pop; the
        # rest of the loads and the bias build follow in the queue
        deferred.append(lambda: load_wp(0))
        deferred.append(lambda: load_wp(1))
        group_scores(0, qk0, nacc0)
        for h in range(2, H):
            deferred.append(lambda h=h: load_wp(h))
        deferred.append(lambda: build_bias(0))
        deferred.append(lambda: build_bias(1))

        for g in range(1, G):
            qk_sb = qk_pool.tile([P, NB, 2 * GC], BF, tag="qk")
            nacc = naccpool.tile([P, 2 * GC], F32, tag="nacc")
            for nb in range(NB):
                qk_block(g, nb, qk_sb, nacc)
            group_scores(g, qk_sb, nacc)

        # group 3's softmax + Q-build run inline (nothing else to overlap)
        emit_deferred(len(deferred))
        qkctx.close()

        # ---- Phase C: W2 = Wv . Q (per j-chunk, accumulated over heads),
        # then the single output GEMM y = x @ W2 + bias.
        cctx = ctx.enter_context(ExitStack())
        w2_pool = cctx.enter_context(tc.tile_pool(name="w2", bufs=1, side="right"))
        w2 = w2_pool.tile([P, KC, C], BF)
        yout = cctx.enter_context(tc.tile_pool(name="yout", bufs=3, side="right"))

        softctx.close()
        w2ps = cctx.enter_context(tc.tile_pool(name="w2ps", bufs=2, space="PSUM",
                                               side="right"))
        yps = cctx.enter_context(tc.tile_pool(name="yps", bufs=2, space="PSUM",
                                              side="right"))

        for jkc in range(KC):
            w2a = w2ps.tile([P, 512], F32, tag="w2a")
            w2b = w2ps.tile([P, 256], F32, tag="w2b")
            for h in range(H):
                nc.tensor.matmul(w2a, wvT[:, jkc, h, :], q_sb[:, h, 0:512],
                                 start=(h == 0), stop=(h == H - 1))
                nc.tensor.matmul(w2b, wvT[:, jkc, h, :], q_sb[:, h, 512:C],
                                 start=(h == 0), stop=(h == H - 1))
            nc.vector.tensor_copy(w2[:, jkc, 0:512], w2a)
            nc.vector.tensor_copy(w2[:, jkc, 512:C], w2b)

        for nb in range(NB):
            y1 = yps.tile([P, 512], F32, tag="y1")
            y2 = yps.tile([P, 256], F32, tag="y2")
            for kc in range(KC):
                nc.tensor.matmul(y1, xT[:, kc, nb * P:(nb + 1) * P],
                                 w2[:, kc, 0:512],
                                 start=(kc == 0), stop=(kc == KC - 1))
            for kc in range(KC):
                nc.tensor.matmul(y2, xT[:, kc, nb * P:(nb + 1) * P],
                                 w2[:, kc, 512:C],
                                 start=(kc == 0), stop=(kc == KC - 1))
            ysb = yout.tile([P, C], F32, tag="y")
            nc.vector.tensor_add(ysb[:, 0:512], y1, bias_sb[:, 0:512])
            nc.vector.tensor_add(ysb[:, 512:C], y2, bias_sb[:, 512:C])
            nc.sync.dma_start(out=y_d[nb * P:(nb + 1) * P, :], in_=ysb)

        cctx.close()

    # Split multi-wait sync conditions into EventSemaphore instructions —
    # walrus' ACT/DVE instruction structs encode at most one wait.
    bass_rust.generate_event_semaphores(nc)
    return nc


def _in_maps(x, Wqkv, temperature, Wproj, bproj):
    x = np.asarray(x)  # plain numpy before slicing (inputs may be jax arrays)
    wqkv = np.ascontiguousarray(Wqkv, dtype=np.float32)
    temp = np.ascontiguousarray(temperature, dtype=np.float32).reshape(H)
    wproj = np.ascontiguousarray(Wproj, dtype=np.float32)
    bp = np.ascontiguousarray(bproj, dtype=np.float32)
    return [
        {"x": np.ascontiguousarray(x[b], dtype=np.float32), "Wqkv": wqkv,
         "temperature": temp, "Wproj": wproj, "bproj": bp}
        for b in range(x.shape[0])
    ]


def run(x, Wqkv, temperature, Wproj, bproj, trace=False):
    nc = build_nc()
    in_maps = _in_maps(x, Wqkv, temperature, Wproj, bproj)
    res = run_bass_kernel_spmd(nc, in_maps, core_ids=list(range(len(in_maps))),
                               trace=trace)
    out = np.stack([res.results[b]["y"] for b in range(len(in_maps))], axis=0)
    return out.astype(np.float32), res


def kernel(x, Wqkv, temperature, Wproj, bproj):
    out, _ = run(x, Wqkv, temperature, Wproj, bproj, trace=False)
    return out
